# revision 28
# baseline (speedup 1.0000x reference)
# ACCon supervised-contrastive loss on 8 TRN2 NeuronCores (Bass/Tile).
#
# Math (validated ~1e-7 f32 / ~2e-5 bf16 against the jax reference):
#   n = 4096 anchors (view-major stack), d = 128, labels in [0,100)
#   dot = cf @ cf.T (unit rows; |dot| <= 1+1.3e-6, clip elided via guards)
#   rowmax(clip(dot)) == 1 exactly (diagonal)  -> logits = dot - 1
#   alpha = pi*lab/100; D_ij = cos(alpha_i - alpha_j)  [rank-2 PE matmul from
#     host-precomputed (cos a, sin a)]
#   neg_logit = -(dot*D + sqrt((s0 - dot^2)(s1 - D^2)))  = -(q + r)
#   r via exp(0.5*ln(m)), m = (s0-dot^2)(s1-D^2)  [one ACT table set:
#     natural_log_exp_and_others covers Copy/Ln/Exp/Square]
#   positives (same label, incl diag): select by exact bf16 label compare;
#     branch value 1-u ~= 1-(dot-eps) (r_pos <= 1.4e-3, validated)
#   Z_i = sum_j exp(-u_sel) - 1 ; S_i = sum_j pos*q - pall_i ; P_i = pall_i-1
#   loss_i = (P_i*ln(Z_i) - S_i - tau)/(P_i + tau);  out = mean_i loss_i
#
# Sharding: core c owns rows [c*512, (c+1)*512) (4 row-tiles x 128); full
# [128,4096] feature matrix replicated to every core (no collectives).
# Per (row-tile, col-group W=2048): 4+4 matmuls into two wide PSUM tensors,
# ACT evacuates to bf16, custom DVE ops (M, USEL, SRED) + TT do the rest;
# fused accum_out row-sums feed a tiny per-row epilogue; host averages the
# 8 x [128,4] per-row losses.

import math
import sys
from operator import add as _opadd

import numpy as np

for _p in ("/opt/trn_rl_repo",):
    if _p not in sys.path:
        sys.path.insert(0, _p)

import concourse.bass as bass  # noqa: E402,F401
import concourse.mybir as mybir  # noqa: E402
import concourse.tile as tile  # noqa: E402
from concourse import bacc  # noqa: E402
from concourse import dve_ops as dvo  # noqa: E402
from concourse.bass_utils import run_bass_kernel_spmd  # noqa: E402
from concourse.dve_spec import (  # noqa: E402
    C0,
    C1,
    C2,
    Spec,
    Src0,
    Src1,
    Zero,
    _has_src1,
    eq,
    lower,
    relu,
    select,
    sq,
)
from concourse.dve_table_gen import dve_ver_for  # noqa: E402
from concourse.dve_uop import DveOpSpec  # noqa: E402

try:
    import ml_dtypes

    _BF16_NP = ml_dtypes.bfloat16
except ImportError:  # pragma: no cover
    _BF16_NP = None

F32 = mybir.dt.float32
BF16 = mybir.dt.bfloat16
ALU = mybir.AluOpType
ACTF = mybir.ActivationFunctionType

N = 4096
DIM = 128
NCORES = 8
RPC = N // NCORES  # 512 rows per core
RT = RPC // 128  # 4 row-tiles
W = 2048  # wide column group
NG = N // W  # 2 col groups
MM = W // 512  # matmuls per wide psum tensor

TAU = 1e-6
S0 = 1.0 + TAU + 3e-6
S1 = 0.9995

_CACHE = {}

# scheduling knobs (tuned via TimelineSim sweep)
EVAC_DVE_SLOTS = (2, 4, 6)
UADD_DVE_SLOTS = tuple(range(8))
STARTUP_PIECES = False
WORK_BUFS = 4


# --------------------------------------------------------------------------
# custom DVE ops
def _make_op(name, spec, perf=True):
    if name not in dvo._SUB_OPCODE_FOR_NAME:
        row = max(dvo._SUB_OPCODE_FOR_NAME.values()) + 1
        assert row < 0x20, "no free custom-DVE rows"
        dvo._SUB_OPCODE_FOR_NAME[name] = row
    ver = dve_ver_for("TRN2")
    uops = lower(spec, ver=ver)
    s = DveOpSpec(
        name=name,
        opcode=dvo._SUB_OPCODE_FOR_NAME[name],
        uops=uops,
        rd1_en=_has_src1(spec),
    )
    op = dvo.DveOp(
        name, spec, subdim=False, uops_sha={ver: s.sha(ver)}, perf_en={ver: perf}
    )
    if all(o.name != name for o in dvo.OPS):
        dvo.OPS.append(op)
        dvo.CUSTOM_DVE_SPECS[name] = spec
    return op


def _ref_m(in0, in1, s0, s1, imm2):
    a = s0 - in0.astype(np.float32) ** 2
    b = s1 - in1.astype(np.float32) ** 2
    return np.maximum(a * b, 0.0).astype(np.float32)


def _ref_qmul(in0, in1, s0, s1, imm2):
    b = (in0.astype(np.float32) * in1.astype(np.float32)).astype(np.float32)
    return b, b.reshape(b.shape[0], -1).sum(axis=-1, keepdims=True)


def _ref_sred(in0, in1, s0, s1, imm2):
    x = in0.astype(np.float32)
    b = np.where(in1.astype(np.float32) == s0, x, 0.0).astype(np.float32)
    return b, b.reshape(b.shape[0], -1).sum(axis=-1, keepdims=True)


def _ref_usel3(in0, in1, s0, s1, imm2):
    q = in0.astype(np.float32)
    r = in1.astype(np.float32)
    b = np.where(r < s0, imm2 - q, q + r).astype(np.float32)
    return b, b.reshape(b.shape[0], -1).sum(axis=-1, keepdims=True)


def _register_ops():
    if "ops" in _CACHE:
        return _CACHE["ops"]
    m_op = _make_op(
        "ACC_MR_ANT", Spec(body=relu((C0 - sq(Src0)) * (C1 - sq(Src1))), reference=_ref_m)
    )
    usel_op = _make_op(
        "ACC_USEL3_ANT",
        Spec(
            body=select(Src1 < C0, C2 - Src0, Src0 + Src1),
            accum=_opadd,
            accum_init=Zero,
            reference=_ref_usel3,
        ),
    )
    qmul_op = _make_op(
        "ACC_QMUL_ANT",
        Spec(
            body=Src0 * Src1,
            accum=_opadd,
            accum_init=Zero,
            reference=_ref_qmul,
        ),
    )
    _CACHE["ops"] = (m_op, usel_op, qmul_op)
    return _CACHE["ops"]


def _pin_act_table():
    """Make the ACT funcs we use exclusive to one table set so the inserter
    emits one table load instead of thrashing (~2.7us per load)."""
    import concourse.hw_specs as hw_specs

    tabs = hw_specs.get_activation_tables("gen3")
    keep = "natural_log_exp_and_others"
    mine = {ACTF.Exp, ACTF.Ln, ACTF.Square, ACTF.Copy, ACTF.Identity}
    assert mine <= tabs[keep]
    for k, v in tabs.items():
        if k != keep:
            v -= mine


# --------------------------------------------------------------------------
def _build():
    _pin_act_table()
    m_op, usel_op, qmul_op = _register_ops()
    nc = bacc.Bacc(
        "TRN2",
        target_bir_lowering=False,
        debug=False,
        enable_asserts=False,
        num_devices=NCORES,
    )
    for val in (-1.0, 1e-20):
        t = nc.alloc_sbuf_tensor(f"const-f32-{val}", [128, 1], F32)
        nc.gpsimd.memset(t.ap(), val)
        nc.const_aps.aps[(F32, val)] = t.ap()
    nc.all_engine_barrier()

    ct_all = nc.dram_tensor("ct_all", [DIM, N], BF16, kind="ExternalInput").ap()
    ct_rows = nc.dram_tensor("ct_rows", [DIM, RPC], BF16, kind="ExternalInput").ap()
    cs_all = nc.dram_tensor("cs_all", [8, N], BF16, kind="ExternalInput").ap()
    cs_rows = nc.dram_tensor("cs_rows", [8, RPC], BF16, kind="ExternalInput").ap()
    smalls = nc.dram_tensor("smalls", [128, 2 * RT], F32, kind="ExternalInput").ap()
    out = nc.dram_tensor("out", [128, RT], F32, kind="ExternalOutput").ap()

    with tile.TileContext(nc) as tc:
        with (
            tc.tile_pool(name="consts", bufs=1) as consts,
            tc.tile_pool(name="psum", bufs=1, space="PSUM") as psum,
            tc.tile_pool(name="work", bufs=WORK_BUFS) as work,
        ):
            # ---- constant loads (lhsT + first rhs pieces first) ----
            ctr = consts.tile([DIM, RPC], BF16, tag="ctr")
            nc.sync.dma_start(ctr[:], ct_rows[:])
            csr = consts.tile([8, RPC], BF16, tag="csr")
            nc.sync.dma_start(csr[:], cs_rows[:])
            csa = consts.tile([8, N], BF16, tag="csa")
            nc.sync.dma_start(csa[:], cs_all[:])
            ctab = consts.tile([DIM, N], BF16, tag="ctab")
            for i in range(4):
                nc.sync.dma_start(
                    ctab[:, i * 1024 : (i + 1) * 1024],
                    ct_all[:, i * 1024 : (i + 1) * 1024],
                )
            cta = [ctab[:, g * W : (g + 1) * W] for g in range(NG)]
            sm_sb = consts.tile([128, 2 * RT], F32, tag="sm_sb")
            nc.sync.dma_start(sm_sb[:], smalls[:])
            pall_sb = sm_sb[:, 0:RT]
            pinv_sb = sm_sb[:, RT : 2 * RT]

            # accumulator slots: col = g*RT + rt
            zacc = consts.tile([128, NG * RT], F32, tag="zacc")
            qacc = consts.tile([128, NG * RT], F32, tag="qacc")
            wacc = consts.tile([128, NG * RT], F32, tag="wacc")
            racc = consts.tile([128, NG * RT], F32, tag="racc")

            # ---- main loop: col-group outer, row-tile inner ----
            for g in range(NG):
                for rt in range(RT):
                    slot = g * RT + rt
                    lhs_f = ctr[:, rt * 128 : (rt + 1) * 128]
                    lhs_cs = csr[:, rt * 128 : (rt + 1) * 128]
                    pa = psum.tile([128, W], F32, tag="pa")
                    pb = psum.tile([128, W], F32, tag="pb")
                    for i in range(MM):
                        sl = slice(i * 512, (i + 1) * 512)
                        nc.tensor.matmul(
                            pa[:, sl], lhs_f, cta[g][:, sl], start=True, stop=True
                        )
                        nc.tensor.matmul(
                            pb[:, sl],
                            lhs_cs,
                            csa[:, g * W + i * 512 : g * W + (i + 1) * 512],
                            start=True,
                            stop=True,
                        )
                    # evacuate dot PSUM -> bf16 (D stays in PSUM, read twice);
                    # alternate ACT/DVE to balance engine load
                    ctb = work.tile([128, W], BF16, tag="ctb")
                    if slot == 0 and STARTUP_PIECES:
                        # startup: chase the 4 matmuls with narrow evacs
                        for i in range(MM):
                            nc.scalar.activation(
                                ctb[:, i * 512 : (i + 1) * 512],
                                pa[:, i * 512 : (i + 1) * 512],
                                ACTF.Copy,
                            )
                    elif slot in EVAC_DVE_SLOTS:
                        nc.vector.tensor_scalar_mul(ctb[:], pa[:], 1.0)
                    else:
                        nc.scalar.activation(ctb[:], pa[:], ACTF.Copy)
                    # m = (S0 - ct^2)(S1 - D^2)
                    m = work.tile([128, W], BF16, tag="m")
                    nc.vector._custom_dve(
                        m_op, out=m[:], in0=ctb[:], in1=pb[:], s0=S0, s1=S1
                    )
                    # r = exp(0.5*ln(m))
                    lnm = work.tile([128, W], BF16, tag="lnm")
                    nc.scalar.activation(lnm[:], m[:], ACTF.Ln, bias=1e-20)
                    r = work.tile([128, W], BF16, tag="r")
                    nc.scalar.activation(
                        r[:],
                        lnm[:],
                        ACTF.Exp,
                        scale=0.5,
                        accum_out=racc[:, slot : slot + 1],
                    )
                    # q = ct*D with fused row-sum (for S recovery)
                    q = work.tile([128, W], BF16, tag="q")
                    nc.vector._custom_dve(
                        qmul_op,
                        out=q[:],
                        in0=ctb[:],
                        in1=pb[:],
                        accum_out=qacc[:, slot : slot + 1],
                    )
                    # u = select(r < 1e-10, 1-q, q+r): positives have r ~ 1e-15
                    # (relu/S1 trick), negatives r >= 4e-5 -- r itself is the mask
                    u = work.tile([128, W], BF16, tag="u")
                    nc.vector._custom_dve(
                        usel_op,
                        out=u[:],
                        in0=q[:],
                        in1=r[:],
                        s0=1e-5,
                        imm2=1.0,
                        accum_out=wacc[:, slot : slot + 1],
                    )
                    # Z partial: sum exp(-u)
                    ez = work.tile([128, W], BF16, tag="ez")
                    nc.scalar.activation(
                        ez[:],
                        u[:],
                        ACTF.Exp,
                        scale=-1.0,
                        accum_out=zacc[:, slot : slot + 1],
                    )

            # ---- per-row epilogue ----
            zred = consts.tile([128, RT], F32, tag="zred")
            nc.vector.tensor_add(zred[:], zacc[:, 0:RT], zacc[:, RT : 2 * RT])
            qred = consts.tile([128, RT], F32, tag="qred")
            nc.vector.tensor_add(qred[:], qacc[:, 0:RT], qacc[:, RT : 2 * RT])
            wred = consts.tile([128, RT], F32, tag="wred")
            nc.vector.tensor_add(wred[:], wacc[:, 0:RT], wacc[:, RT : 2 * RT])
            rred = consts.tile([128, RT], F32, tag="rred")
            nc.vector.tensor_add(rred[:], racc[:, 0:RT], racc[:, RT : 2 * RT])
            lz = consts.tile([128, RT], F32, tag="lz")
            nc.scalar.activation(lz[:], zred[:], ACTF.Ln, bias=-1.0)  # ln(Z-1)
            # S = sum_pos q - pall = (Sq + Sr - Su - pall)/2
            qpr = consts.tile([128, RT], F32, tag="qpr")
            nc.vector.tensor_add(qpr[:], qred[:], rred[:])
            qmw = consts.tile([128, RT], F32, tag="qmw")
            nc.vector.tensor_sub(qmw[:], qpr[:], wred[:])
            qmwp = consts.tile([128, RT], F32, tag="qmwp")
            nc.vector.tensor_sub(qmwp[:], qmw[:], pall_sb[:])
            s_t = consts.tile([128, RT], F32, tag="s_t")
            nc.vector.tensor_scalar_mul(s_t[:], qmwp[:], 0.5)
            p_t = consts.tile([128, RT], F32, tag="p_t")
            nc.vector.tensor_scalar_add(p_t[:], pall_sb[:], -1.0)
            pl = consts.tile([128, RT], F32, tag="pl")
            nc.vector.tensor_tensor(pl[:], p_t[:], lz[:], op=ALU.mult)
            num = consts.tile([128, RT], F32, tag="num")
            nc.vector.tensor_sub(num[:], pl[:], s_t[:])
            num2 = consts.tile([128, RT], F32, tag="num2")
            nc.vector.tensor_scalar_add(num2[:], num[:], -TAU)
            res = consts.tile([128, RT], F32, tag="res")
            nc.vector.tensor_tensor(res[:], num2[:], pinv_sb[:], op=ALU.mult)
            nc.sync.dma_start(out[:], res[:])

    nc.compile()
    return nc


def _prep(features: np.ndarray, labels: np.ndarray):
    f = np.asarray(features, dtype=np.float32)
    lab_i = np.asarray(labels, dtype=np.int64)[:, 0]
    cfT = np.ascontiguousarray(f.transpose(2, 1, 0).reshape(DIM, N)).astype(_BF16_NP)
    lab = np.tile(lab_i, 2)
    alpha = lab.astype(np.float64) * (math.pi / 100.0)
    c32 = np.cos(alpha).astype(np.float32)
    s32 = np.sin(alpha).astype(np.float32)

    def _pair(x):
        hi = x.astype(_BF16_NP)
        lo = (x - hi.astype(np.float32)).astype(_BF16_NP)
        return hi, lo

    chi, clo = _pair(c32)
    shi, slo = _pair(s32)
    # lhsT rows and rhs rows pair up so sum_k lhsT[k]*rhs[k] = c*c' + s*s'
    cs_lhs = np.stack([chi, chi, clo, clo, shi, shi, slo, slo]).astype(_BF16_NP)
    cs_rhs = np.stack([chi, clo, chi, clo, shi, slo, shi, slo]).astype(_BF16_NP)
    hist = np.bincount(lab_i, minlength=100)
    pall = np.tile((2.0 * hist[lab_i]).astype(np.float32), 2)
    pinv = (1.0 / (pall - 1.0 + TAU)).astype(np.float32)

    in_maps = []
    for c in range(NCORES):
        rs = slice(c * RPC, (c + 1) * RPC)
        in_maps.append(
            {
                "ct_all": cfT,
                "ct_rows": np.ascontiguousarray(cfT[:, rs]),
                "cs_all": np.ascontiguousarray(cs_rhs),
                "cs_rows": np.ascontiguousarray(cs_lhs[:, rs]),
                "smalls": np.ascontiguousarray(
                    np.concatenate(
                        [
                            pall[rs].reshape(RT, 128).T,
                            pinv[rs].reshape(RT, 128).T,
                        ],
                        axis=1,
                    )
                ),
            }
        )
    return in_maps


def kernel(features: np.ndarray, labels: np.ndarray) -> np.ndarray:
    if "nc" not in _CACHE:
        _CACHE["nc"] = _build()
    nc = _CACHE["nc"]
    in_maps = _prep(features, labels)
    res = run_bass_kernel_spmd(nc, in_maps, core_ids=list(range(NCORES)))
    total = 0.0
    for c in range(NCORES):
        total += float(res.results[c]["out"].sum())
    return np.float32(total / N)


if __name__ == "__main__":
    rng = np.random.default_rng(0)
    feats = rng.normal(size=(2048, 2, 128)).astype(np.float32)
    feats /= np.linalg.norm(feats, axis=-1, keepdims=True)
    labs = rng.integers(0, 100, size=(2048, 1)).astype(np.int32)
    print("loss:", kernel(features=feats, labels=labs))
